# revision 14
# baseline (speedup 1.0000x reference)
"""Trainium2 Bass kernel for nn_DetectionLoss (YOLO-style detection loss).

Strategy (v2)
-------------
Device work per core (SPMD, 8 cores, batch-sharded 2 batches/core):
  - dense objectness: sum softplus over the 18 obj planes (Exp+Ln(1+x) with
    ACT-side accumulation straight into the output tile).
  - sparse CIoU/cls/obj terms on host-pre-gathered target rows, computed as
    a two-lane (DVE + Pool) dependency graph over [128,3,k] tiles with
    sign-packed box algebra:
      B = [p2c, -p1c] via one add; min(B,T4) = [imin, -imax];
      max(B,T4) = [cmax, -cmin]  (T4 = [tx2, ty2, -tx1, -ty1] host-packed)
    so each min/max/widths pair is one instruction.
  - arctan via z-transform (z=(w-h)/(w+h) in [-1,1]) + 3-term odd poly.
  - inputs split across three DMA queues (SP, DVE-HWDGE, Pool-SWDGE); the
    class logits travel as bf16 (rel-tol 2e-2 gives plenty of headroom).
  - no DMA on the ACT queue: an ACT-engine DMA makes bacc insert a second
    activation-table load (set 0) which costs 1.28us of ACT time.

Host does: index math on the tiny [300,6] targets, the sparse gather,
sharding, and the final scalar reduction of the 8x[128,12] partials.
"""

import os
import sys

for _p in ("/opt/trn_rl_repo", "/root/.axon_site/_ro/trn_rl_repo"):
    if os.path.isdir(_p) and _p not in sys.path:
        sys.path.append(_p)

import ml_dtypes
import numpy as np

import concourse.bass as bass
import concourse.tile as tile
from concourse import bacc, mybir
from concourse.bass_utils import run_bass_kernel_spmd

F32 = mybir.dt.float32
BF16 = mybir.dt.bfloat16
AF = mybir.ActivationFunctionType
OP = mybir.AluOpType
AX = mybir.AxisListType

ANCHORS = [[(10, 13), (16, 30), (33, 23)],
           [(30, 61), (62, 45), (59, 119)],
           [(116, 90), (156, 198), (373, 326)]]
STRIDES = [8.0, 16.0, 32.0]
GRIDS = [80, 40, 20]
NUM_CLASSES = 80
LAMBDA_BOX, LAMBDA_OBJ, LAMBDA_CLS = 0.05, 1.0, 0.5
ANCHOR_THRESH = 4.0
EPS = 1e-7
K4PI2 = float(4.0 / np.pi ** 2)
# minimax-ish odd poly for atan(z), z in [-1,1]: ((A2*u + A1)*u + A0)*z, u=z^2
AT0, AT1, AT2 = 0.99570658, -0.29065994, 0.08132491

M = 8          # cores
B = 16         # batch
BPC = B // M   # batches per core
N_TGT = 300
TPC = 38       # targets per core (8*38 = 304 >= 300, padded)
NA = 3         # anchors per scale

# dense obj planes: all three scales map onto 80 partitions
# (6400=80x80, 1600=80x20, 400=80x5) -> one [80, 630] SBUF tile
_DP = 80
_DCOLS = [480, 120, 30]
_DOFF = [0, 480, 600]
_DTOT = 630

# packed fp32 sparse layout [128, 3, NF]
C_OBJ = 0
C_XY4 = 1    # xy logits duplicated: cols 1-4
C_WH4 = 5    # wh logits duplicated: cols 5-8
C_KF = 9
C_M = 10
C_GI4 = 11   # [gi, gj, -gi, -gj]
C_T4 = 15    # [tx2, ty2, -tx1, -ty1]
C_AWS4 = 19  # [awx, awy, awx, awy] = anc/stride/2
C_GT = 23    # [tsxh - gi, tsyh - gj]
C_ATANT = 25  # atan_t - pi/4
C_AREA2 = 26  # pad 1.0
C_LCLS = 27
NF = 28

# module-level caches (compile once per process)
_NC = None
LAST_EXEC_TIME_NS = None
LAST_RESULT = None


def _build_program():
    nc = bacc.Bacc(None, enable_partition_id=False, detect_race_conditions=False)
    p0d = nc.dram_tensor("p0", [BPC * 255, 6400], F32, kind="ExternalInput")
    p1d = nc.dram_tensor("p1", [BPC * 255, 1600], F32, kind="ExternalInput")
    p2d = nc.dram_tensor("p2", [BPC * 255, 400], F32, kind="ExternalInput")
    spfd = nc.dram_tensor("spf", [128, 3, NF], F32, kind="ExternalInput")
    spcd = nc.dram_tensor("spc", [128, 3, NUM_CLASSES], BF16, kind="ExternalInput")
    outd = nc.dram_tensor("out", [128, 12], F32, kind="ExternalOutput")

    from concourse.tile_rust import add_dep_helper

    # per-engine scheduling chains (order only, no extra semaphores)
    chains = {}

    def chained(key, ins):
        if key in chains:
            add_dep_helper(ins.ins, chains[key].ins, sync=False,
                           reason=f"{key} order")
        chains[key] = ins
        return ins

    with tile.TileContext(nc) as tc:
        with tc.tile_pool(name="sb", bufs=1) as pool:
            uid = [0]

            def mk(shape, nm, dtype=F32):
                uid[0] += 1
                return pool.tile(shape, dtype, name=f"{nm}{uid[0]}",
                                 tag=f"{nm}{uid[0]}")

            V = nc.vector
            G = nc.gpsimd
            S = nc.scalar

            def gp(ins):
                return chained("pool", ins)

            def dv(ins):
                return chained("dve", ins)

            def ac(ins):
                return chained("act", ins)

            # ---------------- tiles ----------------
            out_t = mk([128, 12], "out_t")
            spf = mk([128, 3, NF], "spf")
            spc = mk([128, 3, NUM_CLASSES], "spc", BF16)
            ddt = mk([_DP, _DTOT], "ddt")
            m4 = mk([128, 3, 4], "m4")
            m2 = mk([128, 3, 2], "m2")

            # ---------------- Pool preamble: constants ----------------
            gp(G.memset(out_t[:], 0.0))
            gp(G.memset(m4[:, :, 0:2], 1.0))
            gp(G.memset(m4[:, :, 2:4], -1.0))
            gp(G.memset(m2[:, :, 0:1], -1.0))
            gp(G.memset(m2[:, :, 1:2], 1.0))

            # ---------------- DMAs (2 queues: SP HWDGE + Pool SWDGE) -----
            # Q_SP: sparse fp32 pack, then dense p0 lower half / p1 / p2
            chained("sp", nc.sync.dma_start(out=spf[:], in_=spfd[:]))
            chained("sp", nc.sync.dma_start(
                out=ddt[:, 0:240].rearrange("p (c f) -> p c f", c=3),
                in_=p0d[0:255:85, :].rearrange("c (p f) -> p c f", p=_DP)))
            chained("sp", nc.sync.dma_start(
                out=ddt[:, 480:600].rearrange("p (c f) -> p c f", c=6),
                in_=p1d[::85, :].rearrange("c (p f) -> p c f", p=_DP)))
            chained("sp", nc.sync.dma_start(
                out=ddt[:, 600:630].rearrange("p (c f) -> p c f", c=6),
                in_=p2d[::85, :].rearrange("c (p f) -> p c f", p=_DP)))
            # Q_POOL (SWDGE): cls bf16, then dense p0 upper half.
            # Chained after the memsets and BEFORE whc4: the ~1us-per-DMA
            # SWDGE descriptor generation must not wait behind compute that
            # blocks on the spf DMA.
            gp(G.dma_start(out=spc[:], in_=spcd[:]))
            gp(G.dma_start(
                out=ddt[:, 240:480].rearrange("p (c f) -> p c f", c=3),
                in_=p0d[255:510:85, :].rearrange("c (p f) -> p c f", p=_DP)))

            def nt(nm, k=1):
                return mk([128, 3] if k == 1 else [128, 3, k], nm)

            # ---------------- Pool: clip wh (x2 dup) ----------------
            whc4 = nt("whc4", 4)
            gp(G.tensor_scalar(whc4[:], spf[:, :, C_WH4:C_WH4 + 4], 4.0, -4.0,
                               op0=OP.min, op1=OP.max))

            # ---------------- ACT sequence (single Exp/Ln table) --------
            exn4 = nt("exn4", 4)
            ac(S.activation(exn4[:], spf[:, :, C_XY4:C_XY4 + 4], AF.Exp,
                            scale=-1.0))
            ewh4 = nt("ewh4", 4)
            ac(S.activation(ewh4[:], whc4[:], AF.Exp))
            ecl = mk([128, 3, NUM_CLASSES], "ecl")
            ac(S.activation(ecl[:], spc[:], AF.Exp))
            lcl = mk([128, 3, NUM_CLASSES], "lcl")
            ac(S.activation(lcl[:], ecl[:], AF.Ln, bias=1.0))
            # dense: per-scale Exp then Ln with accumulation into out_t cols
            for s in range(3):
                dex = mk([_DP, _DCOLS[s]], "dex")
                ac(S.activation(dex[:], ddt[:, _DOFF[s]:_DOFF[s] + _DCOLS[s]],
                                AF.Exp))
                dlt = mk([_DP, _DCOLS[s]], "dlt")
                ac(S.activation(dlt[:], dex[:], AF.Ln, bias=1.0,
                                accum_out=out_t[0:_DP, s:s + 1]))

            # ---------------- two-lane sparse chain ----------------
            # DVE head: sigmoid pair [sg, -sg] then box corners
            d14 = nt("d14", 4)
            dv(V.scalar_tensor_tensor(d14[:], exn4[:], 1.0, m4[:],
                                      OP.add, OP.mult))
            sg4 = nt("sg4", 4)
            dv(V.reciprocal(sg4[:], d14[:]))

            # Pool: H4 = ewh*aws (half-widths dup), GH = GI4 + H4
            H4 = nt("H4", 4)
            gp(G.tensor_tensor(H4[:], ewh4[:], spf[:, :, C_AWS4:C_AWS4 + 4],
                               OP.mult))
            GH = nt("GH", 4)
            gp(G.tensor_tensor(GH[:], spf[:, :, C_GI4:C_GI4 + 4], H4[:],
                               OP.add))
            # arctan z inputs: HH2 = hh*[-1,1], ND = [hw,hw] + HH2
            # (the reference's eps in w/(h+eps) is dropped: h >= e^-4*aw/s/2)
            HH2 = nt("HH2", 2)
            gp(G.tensor_tensor(HH2[:], H4[:, :, 1::2], m2[:], OP.mult))
            ND = nt("ND", 2)
            gp(G.tensor_tensor(ND[:], H4[:, :, 0::2], HH2[:], OP.add))
            # dc' = -sg + (tsxh - gi)  (negated center offset; squared later)
            dcp = nt("dcp", 2)
            gp(G.tensor_tensor(dcp[:], sg4[:, :, 2:4],
                               spf[:, :, C_GT:C_GT + 2], OP.add))
            # quarter-area hw*hh (x4 and +area2+eps folded in on DVE below)
            area1 = nt("area1")
            gp(G.tensor_tensor(area1[:], H4[:, :, 0], H4[:, :, 1], OP.mult))

            # DVE: B = sg4 + GH = [p2c, -p1c]
            Bx = nt("Bx", 4)
            dv(V.tensor_tensor(Bx[:], sg4[:], GH[:], OP.add))
            I4 = nt("I4", 4)
            dv(V.tensor_tensor(I4[:], Bx[:], spf[:, :, C_T4:C_T4 + 4], OP.min))
            rdn = nt("rdn")
            dv(V.reciprocal(rdn[:], ND[:, :, 1]))
            z = nt("z")
            dv(V.tensor_tensor(z[:], ND[:, :, 0], rdn[:], OP.mult))
            iwh = nt("iwh", 2)
            dv(V.tensor_tensor(iwh[:], I4[:, :, 0:2], I4[:, :, 2:4], OP.add))
            iwc = nt("iwc", 2)
            dv(V.tensor_scalar_max(iwc[:], iwh[:], 0.0))
            inter = nt("inter")
            dv(V.tensor_tensor(inter[:], iwc[:, :, 0], iwc[:, :, 1], OP.mult))
            # u1n = -(union) = inter - (area2+eps) - 4*hw*hh
            w_ = nt("w_")
            dv(V.tensor_tensor(w_[:], inter[:], spf[:, :, C_AREA2],
                               OP.subtract))
            u1n = nt("u1n")
            dv(V.affine_then_add(u1n[:], area1[:], w_[:], -4.0, 0.0))
            run_ = nt("run")
            dv(V.reciprocal(run_[:], u1n[:]))
            ioun = nt("ioun")
            dv(V.tensor_tensor(ioun[:], inter[:], run_[:], OP.mult))

            # Pool: enclosing box via max(a,b) = a+b-min(a,b) (Pool has no
            # tensor-tensor min/max): cwh = (B+T4 summed pairs) - iwh_raw
            bt = nt("bt", 4)
            gp(G.tensor_tensor(bt[:], Bx[:], spf[:, :, C_T4:C_T4 + 4], OP.add))
            btw = nt("btw", 2)
            gp(G.tensor_tensor(btw[:], bt[:, :, 0:2], bt[:, :, 2:4], OP.add))
            cwh = nt("cwh", 2)
            gp(G.tensor_tensor(cwh[:], btw[:], iwh[:], OP.subtract))
            csq = nt("csq", 2)
            gp(G.tensor_tensor(csq[:], cwh[:], cwh[:], OP.mult))
            c2t = nt("c2t")
            gp(G.tensor_tensor(c2t[:], csq[:, :, 0], csq[:, :, 1], OP.add))
            dsq = nt("dsq", 2)
            gp(G.tensor_tensor(dsq[:], dcp[:], dcp[:], OP.mult))
            rho = nt("rho")
            gp(G.tensor_tensor(rho[:], dsq[:, :, 0], dsq[:, :, 1], OP.add))
            pu = nt("pu")
            gp(G.tensor_tensor(pu[:], z[:], z[:], OP.mult))
            pb = nt("pb")
            gp(G.tensor_scalar(pb[:], pu[:], AT2, AT1, op0=OP.mult, op1=OP.add))
            pc_ = nt("pc_")
            gp(G.tensor_tensor(pc_[:], pb[:], pu[:], OP.mult))
            pd_ = nt("pd_")
            gp(G.tensor_scalar(pd_[:], pc_[:], AT0, None, op0=OP.add))
            at = nt("at")
            gp(G.tensor_tensor(at[:], pd_[:], z[:], OP.mult))
            dat = nt("dat")
            gp(G.tensor_tensor(dat[:], spf[:, :, C_ATANT], at[:],
                               OP.subtract))
            q = nt("q")
            gp(G.tensor_tensor(q[:], dat[:], dat[:], OP.mult))
            q2 = nt("q2")
            gp(G.tensor_tensor(q2[:], q[:], q[:], OP.mult))

            # DVE: rc2 for rho2/c2 and the cls reduce (DVE-only op)
            rc2 = nt("rc2")
            dv(V.reciprocal(rc2[:], c2t[:]))
            csum = nt("csum")
            dv(V.tensor_reduce(csum[:], lcl[:], AX.X, op=OP.add))
            trho = nt("trho")
            gp(G.tensor_tensor(trho[:], rho[:], rc2[:], OP.mult))

            # Pool: cls tail + outputs 6..12
            csub = nt("csub")
            gp(G.tensor_tensor(csub[:], csum[:], spf[:, :, C_LCLS],
                               OP.subtract))
            gp(G.tensor_tensor(out_t[:, 9:12], csub[:], spf[:, :, C_KF],
                               OP.mult))
            gp(G.tensor_tensor(out_t[:, 6:9], spf[:, :, C_OBJ],
                               spf[:, :, C_M], OP.mult))

            # DVE tail: ta = (1 - iou) + trho; s1 = K*q + (1+eps) - iou
            ta = nt("ta")
            dv(V.affine_then_add(ta[:], ioun[:], trho[:], 1.0, 1.0))
            s1 = nt("s1")
            dv(V.affine_then_add(s1[:], q[:], ioun[:], K4PI2, 1.0 + EPS))
            rd = nt("rd")
            dv(V.reciprocal(rd[:], s1[:]))
            va = nt("va")
            dv(V.scalar_tensor_tensor(va[:], q2[:], K4PI2 * K4PI2, rd[:],
                                      OP.mult, OP.mult))
            tb = nt("tb")
            dv(V.tensor_tensor(tb[:], ta[:], va[:], OP.add))
            dv(V.tensor_tensor(out_t[:, 3:6], tb[:], spf[:, :, C_KF],
                               OP.mult))

            nc.sync.dma_start(out=outd[:], in_=out_t[:])

    # Keep Exp/Ln confined to one activation table so only one
    # ACT_TABLE_LOAD is emitted (see baseline comment).
    from concourse.hw_specs import get_activation_tables
    orig_tables = get_activation_tables(nc.m.arch)
    tweaked = {}
    for name, fns in orig_tables.items():
        fns = set(fns)
        if name != "natural_log_exp_and_others":
            fns.discard(AF.Exp)
            fns.discard(AF.Ln)
        tweaked[name] = fns
    orig_fn = bacc.get_activation_tables
    bacc.get_activation_tables = lambda arch: tweaked
    try:
        nc.compile()
    finally:
        bacc.get_activation_tables = orig_fn
    return nc


def _get_program():
    global _NC
    if _NC is None:
        _NC = _build_program()
    return _NC


def _prep_host(p0, p1, p2, targets, img_size):
    """Index math, anchor matching, gather and per-core packing (numpy)."""
    t = np.ascontiguousarray(targets, dtype=np.float32)
    img = np.float32(img_size)
    bi = t[:, 0].astype(np.int32)
    cls = t[:, 1].astype(np.int32)
    preds = [np.ascontiguousarray(p, dtype=np.float32) for p in (p0, p1, p2)]

    spf_all = np.zeros((M, 128, 3, NF), np.float32)
    spc_all = np.zeros((M, 128, 3, NUM_CLASSES), ml_dtypes.bfloat16)
    # pad-row defaults keeping device math finite (kf=m=0 contribute nothing)
    spf_all[..., C_T4 + 0] = 1.0
    spf_all[..., C_T4 + 1] = 1.0
    spf_all[..., C_AREA2] = 1.0
    spf_all[..., C_AWS4:C_AWS4 + 4] = 0.5  # keeps hh>0 so z stays finite

    nkeep = []
    counts = []
    for s in range(3):
        Gr = GRIDS[s]
        stride = np.float32(STRIDES[s])
        anc = np.asarray(ANCHORS[s], dtype=np.float32)  # [3,2]
        gt_wh = t[:, 4:6] * img
        r = gt_wh[None, :, :] / anc[:, None, :]
        rr = np.maximum(r, np.float32(1.0) / np.clip(r, np.float32(1e-8), None))
        keep = rr.max(-1) < np.float32(ANCHOR_THRESH)  # [3,N]
        kf = keep.astype(np.float32)
        nkeep.append(float(np.maximum(kf.sum(dtype=np.float32), np.float32(1.0))))
        counts.append(float(B * NA * Gr * Gr))

        Gf = np.float32(Gr)
        cx = t[:, 2] * Gf
        cy = t[:, 3] * Gf
        gw = t[:, 4] * Gf
        gh = t[:, 5] * Gf
        gi = np.clip(cx.astype(np.int32), 0, Gr - 1)
        gj = np.clip(cy.astype(np.int32), 0, Gr - 1)
        tx1 = cx - gw / 2
        ty1 = cy - gh / 2
        tx2 = cx + gw / 2
        ty2 = cy + gh / 2
        w2p = (tx2 - tx1) * stride
        h2p = (ty2 - ty1) * stride
        atan_t = np.arctan(w2p / (h2p + np.float32(EPS)))
        area2 = (tx2 - tx1) * (ty2 - ty1)
        tsxh = (tx1 + tx2) * np.float32(0.5)
        tsyh = (ty1 + ty2) * np.float32(0.5)

        # dedup mask for the objectness scatter
        mrep = np.zeros((NA, N_TGT), np.float32)
        seen = set()
        for a in range(NA):
            for n in np.nonzero(keep[a])[0]:
                key = (int(bi[n]), a, int(gj[n]), int(gi[n]))
                if key not in seen:
                    seen.add(key)
                    mrep[a, n] = 1.0

        gat = preds[s][bi, :, gj, gi].reshape(N_TGT, NA, 85)  # [N,3,85]
        lcls = gat[np.arange(N_TGT)[:, None], np.arange(NA)[None, :],
                   (5 + cls)[:, None]]  # [N,3]

        for i in range(M):
            n0 = i * TPC
            n1 = min(n0 + TPC, N_TGT)
            c = n1 - n0
            if c <= 0:
                continue
            for a in range(NA):
                rows = slice(a * TPC, a * TPC + c)
                g = gat[n0:n1, a, :]
                spf_all[i, rows, s, C_OBJ] = g[:, 0]
                spf_all[i, rows, s, C_XY4:C_XY4 + 2] = g[:, 1:3]
                spf_all[i, rows, s, C_XY4 + 2:C_XY4 + 4] = g[:, 1:3]
                spf_all[i, rows, s, C_WH4:C_WH4 + 2] = g[:, 3:5]
                spf_all[i, rows, s, C_WH4 + 2:C_WH4 + 4] = g[:, 3:5]
                spc_all[i, rows, s, :] = g[:, 5:85]
                spf_all[i, rows, s, C_KF] = kf[a, n0:n1]
                spf_all[i, rows, s, C_M] = mrep[a, n0:n1]
                spf_all[i, rows, s, C_GI4 + 0] = gi[n0:n1]
                spf_all[i, rows, s, C_GI4 + 1] = gj[n0:n1]
                spf_all[i, rows, s, C_GI4 + 2] = -gi[n0:n1]
                spf_all[i, rows, s, C_GI4 + 3] = -gj[n0:n1]
                spf_all[i, rows, s, C_T4 + 0] = tx2[n0:n1]
                spf_all[i, rows, s, C_T4 + 1] = ty2[n0:n1]
                spf_all[i, rows, s, C_T4 + 2] = -tx1[n0:n1]
                spf_all[i, rows, s, C_T4 + 3] = -ty1[n0:n1]
                aw = anc[a, 0] / stride / 2
                ah = anc[a, 1] / stride / 2
                spf_all[i, rows, s, C_AWS4 + 0] = aw
                spf_all[i, rows, s, C_AWS4 + 1] = ah
                spf_all[i, rows, s, C_AWS4 + 2] = aw
                spf_all[i, rows, s, C_AWS4 + 3] = ah
                spf_all[i, rows, s, C_GT + 0] = tsxh[n0:n1] - gi[n0:n1]
                spf_all[i, rows, s, C_GT + 1] = tsyh[n0:n1] - gj[n0:n1]
                spf_all[i, rows, s, C_ATANT] = (atan_t[n0:n1]
                                                - np.float32(np.pi / 4))
                spf_all[i, rows, s, C_AREA2] = area2[n0:n1] + np.float32(EPS)
                spf_all[i, rows, s, C_LCLS] = lcls[n0:n1, a]

    in_maps = []
    for i in range(M):
        in_maps.append({
            "p0": preds[0][BPC * i:BPC * (i + 1)].reshape(BPC * 255, 6400),
            "p1": preds[1][BPC * i:BPC * (i + 1)].reshape(BPC * 255, 1600),
            "p2": preds[2][BPC * i:BPC * (i + 1)].reshape(BPC * 255, 400),
            "spf": np.ascontiguousarray(spf_all[i]),
            "spc": np.ascontiguousarray(spc_all[i]),
        })
    return in_maps, nkeep, counts


def _combine(outs, nkeep, counts):
    """outs: [M,128,12] per-core partials -> final scalar loss."""
    col = outs.sum(axis=(0, 1), dtype=np.float64)
    loss = 0.0
    for s in range(3):
        loss += LAMBDA_BOX * col[3 + s] / nkeep[s]
        loss += LAMBDA_OBJ * (col[s] - col[6 + s]) / counts[s]
        loss += LAMBDA_CLS * col[9 + s] / (nkeep[s] * NUM_CLASSES)
    return np.float32(loss)


def kernel(p0, p1, p2, targets, img_size):
    global LAST_EXEC_TIME_NS, LAST_RESULT
    in_maps, nkeep, counts = _prep_host(p0, p1, p2, targets, img_size)
    nc = _get_program()
    res = run_bass_kernel_spmd(nc, in_maps, core_ids=list(range(M)))
    LAST_EXEC_TIME_NS = getattr(res, "exec_time_ns", None)
    LAST_RESULT = res
    outs = np.stack([r["out"] for r in res.results])
    return _combine(outs, nkeep, counts)


# revision 25
# speedup vs baseline: 1.0160x; 1.0160x over previous
"""Trainium2 Bass kernel for nn_DetectionLoss (YOLO-style detection loss).

Strategy (v2)
-------------
Device work per core (SPMD, 8 cores, batch-sharded 2 batches/core):
  - dense objectness: sum softplus over the 18 obj planes (Exp+Ln(1+x) with
    ACT-side accumulation straight into the output tile).
  - sparse CIoU/cls/obj terms on host-pre-gathered target rows, computed as
    a two-lane (DVE + Pool) dependency graph over [128,3,k] tiles with
    sign-packed box algebra:
      B = [p2c, -p1c] via one add; min(B,T4) = [imin, -imax];
      max(B,T4) = [cmax, -cmin]  (T4 = [tx2, ty2, -tx1, -ty1] host-packed)
    so each min/max/widths pair is one instruction.
  - arctan via z-transform (z=(w-h)/(w+h) in [-1,1]) + 3-term odd poly.
  - inputs split across three DMA queues (SP, DVE-HWDGE, Pool-SWDGE); the
    class logits travel as bf16 (rel-tol 2e-2 gives plenty of headroom).
  - no DMA on the ACT queue: an ACT-engine DMA makes bacc insert a second
    activation-table load (set 0) which costs 1.28us of ACT time.

Host does: index math on the tiny [300,6] targets, the sparse gather,
sharding, and the final scalar reduction of the 8x[128,12] partials.
"""

import os
import sys

for _p in ("/opt/trn_rl_repo", "/root/.axon_site/_ro/trn_rl_repo"):
    if os.path.isdir(_p) and _p not in sys.path:
        sys.path.append(_p)

import ml_dtypes
import numpy as np

import concourse.bass as bass
import concourse.tile as tile
from concourse import bacc, mybir
from concourse.bass_utils import run_bass_kernel_spmd

F32 = mybir.dt.float32
BF16 = mybir.dt.bfloat16
AF = mybir.ActivationFunctionType
OP = mybir.AluOpType
AX = mybir.AxisListType

ANCHORS = [[(10, 13), (16, 30), (33, 23)],
           [(30, 61), (62, 45), (59, 119)],
           [(116, 90), (156, 198), (373, 326)]]
STRIDES = [8.0, 16.0, 32.0]
GRIDS = [80, 40, 20]
NUM_CLASSES = 80
LAMBDA_BOX, LAMBDA_OBJ, LAMBDA_CLS = 0.05, 1.0, 0.5
ANCHOR_THRESH = 4.0
EPS = 1e-7
K4PI2 = float(4.0 / np.pi ** 2)
# minimax-ish odd poly for atan(z), z in [-1,1]: ((A2*u + A1)*u + A0)*z, u=z^2
AT0, AT1, AT2 = 0.99570658, -0.29065994, 0.08132491

M = 8          # cores
B = 16         # batch
BPC = B // M   # batches per core
N_TGT = 300
TPC = 38       # targets per core (8*38 = 304 >= 300, padded)
NA = 3         # anchors per scale

# dense obj planes: partition-single dst tiles [P, 6, F] chosen for
# larger DMA descriptors than an 80-partition layout would give:
# s0: 400B descs, s1/s2: 320B descs
_DSH = [(64, 100), (20, 80), (5, 80)]  # (partitions, floats per desc)

# packed fp32 sparse layout [128, 3, NF]
C_OBJ = 0
C_XY4 = 1    # xy logits duplicated: cols 1-4
C_WH4 = 5    # wh logits duplicated: cols 5-8
C_KF = 9
C_M = 10
C_GI4 = 11   # [gi, gj, -gi, -gj]
C_T4 = 15    # [tx2, ty2, -tx1, -ty1]
C_AWS4 = 19  # [awx, awy, awx, awy] = anc/stride/2
C_GT = 23    # [tsxh - gi, tsyh - gj]
C_ATANT = 25  # atan_t - pi/4
C_AREA2 = 26  # pad 1.0
C_LCLS = 27
NF = 28

# module-level caches (compile once per process)
_NC = None
LAST_EXEC_TIME_NS = None
LAST_RESULT = None


def _build_program():
    nc = bacc.Bacc(None, enable_partition_id=False, detect_race_conditions=False)
    p0d = nc.dram_tensor("p0", [BPC * 255, 6400], F32, kind="ExternalInput")
    p1d = nc.dram_tensor("p1", [BPC * 255, 1600], F32, kind="ExternalInput")
    p2d = nc.dram_tensor("p2", [BPC * 255, 400], F32, kind="ExternalInput")
    spfd = nc.dram_tensor("spf", [128, 3, NF], F32, kind="ExternalInput")
    spcd = nc.dram_tensor("spc", [128, 3, NUM_CLASSES], BF16, kind="ExternalInput")
    outd = nc.dram_tensor("out", [128, 12], F32, kind="ExternalOutput")

    from concourse.tile_rust import add_dep_helper

    # per-engine scheduling chains (order only, no extra semaphores)
    chains = {}

    def chained(key, ins):
        if key in chains:
            add_dep_helper(ins.ins, chains[key].ins, sync=False,
                           reason=f"{key} order")
        chains[key] = ins
        return ins

    with tile.TileContext(nc) as tc:
        with tc.tile_pool(name="sb", bufs=1) as pool:
            uid = [0]

            def mk(shape, nm, dtype=F32):
                uid[0] += 1
                return pool.tile(shape, dtype, name=f"{nm}{uid[0]}",
                                 tag=f"{nm}{uid[0]}")

            V = nc.vector
            G = nc.gpsimd
            S = nc.scalar

            def gp(ins):
                return chained("pool", ins)

            def dv(ins):
                return chained("dve", ins)

            def ac(ins):
                return chained("act", ins)

            # ---------------- tiles ----------------
            out_t = mk([128, 12], "out_t")
            spf = mk([128, 3, NF], "spf")
            spc = mk([128, 3, NUM_CLASSES], "spc", BF16)
            d0 = mk([_DSH[0][0], 6, _DSH[0][1]], "d0")
            d1 = mk([_DSH[1][0], 6, _DSH[1][1]], "d1")
            d2 = mk([_DSH[2][0], 6, _DSH[2][1]], "d2")
            m4 = mk([128, 3, 4], "m4")
            m2 = mk([128, 3, 2], "m2")

            # ---------------- Pool preamble: constants ----------------
            gp(G.memset(out_t[:], 0.0))
            gp(G.memset(m4[:, :, 0:2], 1.0))
            gp(G.memset(m4[:, :, 2:4], -1.0))
            gp(G.memset(m2[:, :, 0:1], -1.0))
            gp(G.memset(m2[:, :, 1:2], 1.0))

            # ---------------- DMAs (2 queues: SP + Pool) ----------------
            # plane-major dense layout -> 200-1600B descriptors.
            # Q_SP: sparse fp32 pack, dense p0 lower half / p1 / p2
            chained("sp", nc.sync.dma_start(out=spf[:], in_=spfd[:]))
            chained("sp", nc.sync.dma_start(
                out=d0[:, 0:3, :],
                in_=p0d[0:255:85, :].rearrange("c (p f) -> p c f",
                                               p=_DSH[0][0])))
            chained("sp", nc.sync.dma_start(
                out=d1[:],
                in_=p1d[::85, :].rearrange("c (p f) -> p c f", p=_DSH[1][0])))
            chained("sp", nc.sync.dma_start(
                out=d2[:],
                in_=p2d[::85, :].rearrange("c (p f) -> p c f", p=_DSH[2][0])))
            # Q_POOL: cls bf16, then dense p0 upper half. Chained after the
            # memsets and BEFORE Pool compute so the issue cost (~1.5us)
            # does not block on the spf DMA.
            gp(G.dma_start(out=spc[:], in_=spcd[:]))
            gp(G.dma_start(
                out=d0[:, 3:6, :],
                in_=p0d[255:510:85, :].rearrange("c (p f) -> p c f",
                                                 p=_DSH[0][0])))

            def nt(nm, k=1):
                return mk([128, 3] if k == 1 else [128, 3, k], nm)

            # ---------------- DVE: clip wh (x2 dup) ----------------
            # (on DVE: Pool is still issuing its DMAs when spf lands)
            whc4 = nt("whc4", 4)
            dv(V.tensor_scalar(whc4[:], spf[:, :, C_WH4:C_WH4 + 4], 4.0, -4.0,
                               op0=OP.min, op1=OP.max))

            # ---------------- ACT sequence (single Exp/Ln table) --------
            exn4 = nt("exn4", 4)
            ac(S.activation(exn4[:], spf[:, :, C_XY4:C_XY4 + 4], AF.Exp,
                            scale=-1.0))
            ewh4 = nt("ewh4", 4)
            ac(S.activation(ewh4[:], whc4[:], AF.Exp))
            ecl = mk([128, 3, NUM_CLASSES], "ecl")
            ac(S.activation(ecl[:], spc[:], AF.Exp))
            lcl = mk([128, 3, NUM_CLASSES], "lcl")
            ac(S.activation(lcl[:], ecl[:], AF.Ln, bias=1.0))
            # dense: per-scale Exp then Ln with accumulation into out_t cols
            for s, dt_ in enumerate((d0, d1, d2)):
                dp, df = _DSH[s]
                dex = mk([dp, 6 * df], "dex")
                ac(S.activation(dex[:], dt_[:], AF.Exp))
                dlt = mk([dp, 6 * df], "dlt")
                ac(S.activation(dlt[:], dex[:], AF.Ln, bias=1.0,
                                accum_out=out_t[0:dp, s:s + 1]))

            # ---------------- two-lane sparse chain ----------------
            # DVE head: sigmoid pair [sg, -sg] then box corners
            d14 = nt("d14", 4)
            dv(V.scalar_tensor_tensor(d14[:], exn4[:], 1.0, m4[:],
                                      OP.add, OP.mult))
            sg4 = nt("sg4", 4)
            dv(V.reciprocal(sg4[:], d14[:]))

            # Pool: obj correction first (needs spf only), then H4/GH
            gp(G.tensor_tensor(out_t[:, 6:9], spf[:, :, C_OBJ],
                               spf[:, :, C_M], OP.mult))
            H4 = nt("H4", 4)
            gp(G.tensor_tensor(H4[:], ewh4[:], spf[:, :, C_AWS4:C_AWS4 + 4],
                               OP.mult))
            GH = nt("GH", 4)
            gp(G.tensor_tensor(GH[:], spf[:, :, C_GI4:C_GI4 + 4], H4[:],
                               OP.add))
            # arctan z inputs: HH2 = hh*[-1,1], ND = [hw,hw] + HH2
            # (the reference's eps in w/(h+eps) is dropped: h >= e^-4*aw/s/2)
            HH2 = nt("HH2", 2)
            gp(G.tensor_tensor(HH2[:], H4[:, :, 1::2], m2[:], OP.mult))
            ND = nt("ND", 2)
            gp(G.tensor_tensor(ND[:], H4[:, :, 0::2], HH2[:], OP.add))
            # dc' = -sg + (tsxh - gi)  (negated center offset; squared later)
            dcp = nt("dcp", 2)
            gp(G.tensor_tensor(dcp[:], sg4[:, :, 2:4],
                               spf[:, :, C_GT:C_GT + 2], OP.add))
            # quarter-area hw*hh (x4 and +area2+eps folded in on DVE below)
            area1 = nt("area1")
            gp(G.tensor_tensor(area1[:], H4[:, :, 0], H4[:, :, 1], OP.mult))

            # DVE: B = sg4 + GH = [p2c, -p1c]
            Bx = nt("Bx", 4)
            dv(V.tensor_tensor(Bx[:], sg4[:], GH[:], OP.add))
            I4 = nt("I4", 4)
            dv(V.tensor_tensor(I4[:], Bx[:], spf[:, :, C_T4:C_T4 + 4], OP.min))
            rdn = nt("rdn")
            dv(V.reciprocal(rdn[:], ND[:, :, 1]))
            z = nt("z")
            dv(V.tensor_tensor(z[:], ND[:, :, 0], rdn[:], OP.mult))
            iwh = nt("iwh", 2)
            dv(V.tensor_tensor(iwh[:], I4[:, :, 0:2], I4[:, :, 2:4], OP.add))
            iwc = nt("iwc", 2)
            dv(V.tensor_scalar_max(iwc[:], iwh[:], 0.0))
            inter = nt("inter")
            dv(V.tensor_tensor(inter[:], iwc[:, :, 0], iwc[:, :, 1], OP.mult))
            # u1n = -(union) = inter - (area2+eps) - 4*hw*hh
            w_ = nt("w_")
            dv(V.tensor_tensor(w_[:], inter[:], spf[:, :, C_AREA2],
                               OP.subtract))
            u1n = nt("u1n")
            dv(V.affine_then_add(u1n[:], area1[:], w_[:], -4.0, 0.0))
            run_ = nt("run")
            dv(V.reciprocal(run_[:], u1n[:]))
            ioun = nt("ioun")
            dv(V.tensor_tensor(ioun[:], inter[:], run_[:], OP.mult))

            # Pool: enclosing box via max(a,b) = a+b-min(a,b) (Pool has no
            # tensor-tensor min/max): cwh = (B+T4 summed pairs) - iwh_raw
            bt = nt("bt", 4)
            gp(G.tensor_tensor(bt[:], Bx[:], spf[:, :, C_T4:C_T4 + 4], OP.add))
            btw = nt("btw", 2)
            gp(G.tensor_tensor(btw[:], bt[:, :, 0:2], bt[:, :, 2:4], OP.add))
            cwh = nt("cwh", 2)
            gp(G.tensor_tensor(cwh[:], btw[:], iwh[:], OP.subtract))
            csq = nt("csq", 2)
            gp(G.tensor_tensor(csq[:], cwh[:], cwh[:], OP.mult))
            c2t = nt("c2t")
            gp(G.tensor_tensor(c2t[:], csq[:, :, 0], csq[:, :, 1], OP.add))
            dsq = nt("dsq", 2)
            gp(G.tensor_tensor(dsq[:], dcp[:], dcp[:], OP.mult))
            rho = nt("rho")
            gp(G.tensor_tensor(rho[:], dsq[:, :, 0], dsq[:, :, 1], OP.add))
            # Pool: arctan poly head (z from DVE); tail (dat,q,q2) on DVE
            pu = nt("pu")
            gp(G.tensor_tensor(pu[:], z[:], z[:], OP.mult))
            pb = nt("pb")
            gp(G.tensor_scalar(pb[:], pu[:], AT2, AT1, op0=OP.mult, op1=OP.add))
            pc_ = nt("pc_")
            gp(G.tensor_tensor(pc_[:], pb[:], pu[:], OP.mult))
            pd_ = nt("pd_")
            gp(G.tensor_scalar(pd_[:], pc_[:], AT0, None, op0=OP.add))
            at = nt("at")
            gp(G.tensor_tensor(at[:], pd_[:], z[:], OP.mult))
            dat = nt("dat")
            dv(V.tensor_tensor(dat[:], spf[:, :, C_ATANT], at[:],
                               OP.subtract))
            q = nt("q")
            dv(V.tensor_tensor(q[:], dat[:], dat[:], OP.mult))
            q2 = nt("q2")
            dv(V.tensor_tensor(q2[:], q[:], q[:], OP.mult))

            # DVE: rc2 for rho2/c2 and the cls reduce (DVE-only op)
            rc2 = nt("rc2")
            dv(V.reciprocal(rc2[:], c2t[:]))
            csum = nt("csum")
            dv(V.tensor_reduce(csum[:], lcl[:], AX.X, op=OP.add))
            trho = nt("trho")
            gp(G.tensor_tensor(trho[:], rho[:], rc2[:], OP.mult))

            # Pool: cls tail
            csub = nt("csub")
            gp(G.tensor_tensor(csub[:], csum[:], spf[:, :, C_LCLS],
                               OP.subtract))
            gp(G.tensor_tensor(out_t[:, 9:12], csub[:], spf[:, :, C_KF],
                               OP.mult))

            # DVE tail: ta = (1 - iou) + trho; s1 = K*q + (1+eps) - iou
            ta = nt("ta")
            dv(V.affine_then_add(ta[:], ioun[:], trho[:], 1.0, 1.0))
            s1 = nt("s1")
            dv(V.affine_then_add(s1[:], q[:], ioun[:], K4PI2, 1.0 + EPS))
            rd = nt("rd")
            dv(V.reciprocal(rd[:], s1[:]))
            va = nt("va")
            dv(V.scalar_tensor_tensor(va[:], q2[:], K4PI2 * K4PI2, rd[:],
                                      OP.mult, OP.mult))
            tb = nt("tb")
            dv(V.tensor_tensor(tb[:], ta[:], va[:], OP.add))
            dv(V.tensor_tensor(out_t[:, 3:6], tb[:], spf[:, :, C_KF],
                               OP.mult))

            nc.sync.dma_start(out=outd[:], in_=out_t[:])

    # Keep Exp/Ln confined to one activation table so only one
    # ACT_TABLE_LOAD is emitted (see baseline comment).
    from concourse.hw_specs import get_activation_tables
    orig_tables = get_activation_tables(nc.m.arch)
    tweaked = {}
    for name, fns in orig_tables.items():
        fns = set(fns)
        if name != "natural_log_exp_and_others":
            fns.discard(AF.Exp)
            fns.discard(AF.Ln)
        tweaked[name] = fns
    orig_fn = bacc.get_activation_tables
    bacc.get_activation_tables = lambda arch: tweaked
    try:
        nc.compile()
    finally:
        bacc.get_activation_tables = orig_fn
    return nc


def _get_program():
    global _NC
    if _NC is None:
        _NC = _build_program()
    return _NC


def _prep_host(p0, p1, p2, targets, img_size):
    """Index math, anchor matching, gather and per-core packing (numpy)."""
    t = np.ascontiguousarray(targets, dtype=np.float32)
    img = np.float32(img_size)
    bi = t[:, 0].astype(np.int32)
    cls = t[:, 1].astype(np.int32)
    preds = [np.ascontiguousarray(p, dtype=np.float32) for p in (p0, p1, p2)]

    spf_all = np.zeros((M, 128, 3, NF), np.float32)
    spc_all = np.zeros((M, 128, 3, NUM_CLASSES), ml_dtypes.bfloat16)
    # pad-row defaults keeping device math finite (kf=m=0 contribute nothing)
    spf_all[..., C_T4 + 0] = 1.0
    spf_all[..., C_T4 + 1] = 1.0
    spf_all[..., C_AREA2] = 1.0
    spf_all[..., C_AWS4:C_AWS4 + 4] = 0.5  # keeps hh>0 so z stays finite

    nkeep = []
    counts = []
    for s in range(3):
        Gr = GRIDS[s]
        stride = np.float32(STRIDES[s])
        anc = np.asarray(ANCHORS[s], dtype=np.float32)  # [3,2]
        gt_wh = t[:, 4:6] * img
        r = gt_wh[None, :, :] / anc[:, None, :]
        rr = np.maximum(r, np.float32(1.0) / np.clip(r, np.float32(1e-8), None))
        keep = rr.max(-1) < np.float32(ANCHOR_THRESH)  # [3,N]
        kf = keep.astype(np.float32)
        nkeep.append(float(np.maximum(kf.sum(dtype=np.float32), np.float32(1.0))))
        counts.append(float(B * NA * Gr * Gr))

        Gf = np.float32(Gr)
        cx = t[:, 2] * Gf
        cy = t[:, 3] * Gf
        gw = t[:, 4] * Gf
        gh = t[:, 5] * Gf
        gi = np.clip(cx.astype(np.int32), 0, Gr - 1)
        gj = np.clip(cy.astype(np.int32), 0, Gr - 1)
        tx1 = cx - gw / 2
        ty1 = cy - gh / 2
        tx2 = cx + gw / 2
        ty2 = cy + gh / 2
        w2p = (tx2 - tx1) * stride
        h2p = (ty2 - ty1) * stride
        atan_t = np.arctan(w2p / (h2p + np.float32(EPS)))
        area2 = (tx2 - tx1) * (ty2 - ty1)
        tsxh = (tx1 + tx2) * np.float32(0.5)
        tsyh = (ty1 + ty2) * np.float32(0.5)

        # dedup mask for the objectness scatter
        mrep = np.zeros((NA, N_TGT), np.float32)
        seen = set()
        for a in range(NA):
            for n in np.nonzero(keep[a])[0]:
                key = (int(bi[n]), a, int(gj[n]), int(gi[n]))
                if key not in seen:
                    seen.add(key)
                    mrep[a, n] = 1.0

        gat = preds[s][bi, :, gj, gi].reshape(N_TGT, NA, 85)  # [N,3,85]
        lcls = gat[np.arange(N_TGT)[:, None], np.arange(NA)[None, :],
                   (5 + cls)[:, None]]  # [N,3]

        for i in range(M):
            n0 = i * TPC
            n1 = min(n0 + TPC, N_TGT)
            c = n1 - n0
            if c <= 0:
                continue
            for a in range(NA):
                rows = slice(a * TPC, a * TPC + c)
                g = gat[n0:n1, a, :]
                spf_all[i, rows, s, C_OBJ] = g[:, 0]
                spf_all[i, rows, s, C_XY4:C_XY4 + 2] = g[:, 1:3]
                spf_all[i, rows, s, C_XY4 + 2:C_XY4 + 4] = g[:, 1:3]
                spf_all[i, rows, s, C_WH4:C_WH4 + 2] = g[:, 3:5]
                spf_all[i, rows, s, C_WH4 + 2:C_WH4 + 4] = g[:, 3:5]
                spc_all[i, rows, s, :] = g[:, 5:85]
                spf_all[i, rows, s, C_KF] = kf[a, n0:n1]
                spf_all[i, rows, s, C_M] = mrep[a, n0:n1]
                spf_all[i, rows, s, C_GI4 + 0] = gi[n0:n1]
                spf_all[i, rows, s, C_GI4 + 1] = gj[n0:n1]
                spf_all[i, rows, s, C_GI4 + 2] = -gi[n0:n1]
                spf_all[i, rows, s, C_GI4 + 3] = -gj[n0:n1]
                spf_all[i, rows, s, C_T4 + 0] = tx2[n0:n1]
                spf_all[i, rows, s, C_T4 + 1] = ty2[n0:n1]
                spf_all[i, rows, s, C_T4 + 2] = -tx1[n0:n1]
                spf_all[i, rows, s, C_T4 + 3] = -ty1[n0:n1]
                aw = anc[a, 0] / stride / 2
                ah = anc[a, 1] / stride / 2
                spf_all[i, rows, s, C_AWS4 + 0] = aw
                spf_all[i, rows, s, C_AWS4 + 1] = ah
                spf_all[i, rows, s, C_AWS4 + 2] = aw
                spf_all[i, rows, s, C_AWS4 + 3] = ah
                spf_all[i, rows, s, C_GT + 0] = tsxh[n0:n1] - gi[n0:n1]
                spf_all[i, rows, s, C_GT + 1] = tsyh[n0:n1] - gj[n0:n1]
                spf_all[i, rows, s, C_ATANT] = (atan_t[n0:n1]
                                                - np.float32(np.pi / 4))
                spf_all[i, rows, s, C_AREA2] = area2[n0:n1] + np.float32(EPS)
                spf_all[i, rows, s, C_LCLS] = lcls[n0:n1, a]

    in_maps = []
    for i in range(M):
        in_maps.append({
            "p0": preds[0][BPC * i:BPC * (i + 1)].reshape(BPC * 255, 6400),
            "p1": preds[1][BPC * i:BPC * (i + 1)].reshape(BPC * 255, 1600),
            "p2": preds[2][BPC * i:BPC * (i + 1)].reshape(BPC * 255, 400),
            "spf": np.ascontiguousarray(spf_all[i]),
            "spc": np.ascontiguousarray(spc_all[i]),
        })
    return in_maps, nkeep, counts


def _combine(outs, nkeep, counts):
    """outs: [M,128,12] per-core partials -> final scalar loss."""
    col = outs.sum(axis=(0, 1), dtype=np.float64)
    loss = 0.0
    for s in range(3):
        loss += LAMBDA_BOX * col[3 + s] / nkeep[s]
        loss += LAMBDA_OBJ * (col[s] - col[6 + s]) / counts[s]
        loss += LAMBDA_CLS * col[9 + s] / (nkeep[s] * NUM_CLASSES)
    return np.float32(loss)


def kernel(p0, p1, p2, targets, img_size):
    global LAST_EXEC_TIME_NS, LAST_RESULT
    in_maps, nkeep, counts = _prep_host(p0, p1, p2, targets, img_size)
    nc = _get_program()
    res = run_bass_kernel_spmd(nc, in_maps, core_ids=list(range(M)))
    LAST_EXEC_TIME_NS = getattr(res, "exec_time_ns", None)
    LAST_RESULT = res
    outs = np.stack([r["out"] for r in res.results])
    return _combine(outs, nkeep, counts)
